# revision 4
# baseline (speedup 1.0000x reference)
"""Bengio-style NNLM forward on 8 Trainium2 NeuronCores (Bass/Tile).

Strategy (vocab-tensor-parallel, per spec sharding hint):
  - The output projection W_2/b_2 and the [B, VOCAB] output are sharded
    across the 8 cores along vocab (6284 columns each, padded from 50257).
  - The small shared part (embedding rows for the batch, fc weights) is
    replicated; each core computes h = tanh(x_e @ W_1.T + b_1) for the
    full batch, then its vocab shard of the output projection.
  - All matmuls run in float32r (fp32 with 11-bit mantissa, full PE rate;
    operands are pre-rounded on host and DMA'd as f32r bit patterns),
    accumulation in fp32 PSUM. Biases: b_e is folded into the gathered
    embedding rows; b_1 via the ACT bias port; b_2 rides as an extra
    x-feature whose value is forced to 1.0 by tanh(0*x + 20) == 1.0f.
  - Host-side prep: shard/pad/transpose weights, gather embedding rows
    (the device then streams the same bytes from HBM), f32r pre-round.
"""
import json
import numpy as np

import concourse.bass as bass
import concourse.mybir as mybir
import concourse.tile as tile
from concourse import bass_utils, bass2jax

F32 = mybir.dt.float32
F32R = mybir.dt.float32r

EMB = 128
CTX = 8
HID = 1000
VOCAB = 50257
B = 4096
N_CORES = 8
VC = 6284            # per-core vocab shard (8 * 6284 = 50272 >= 50257, even)
CHUNK = 1024         # batch chunk
KE, KH = 8, 8        # embed / hidden k-tiles (128 features each)


# ---------------------------------------------------------------------------
# Workaround for walrus builds that allow only ONE sem-wait per instruction:
# split any multi-wait instruction into wait-only EventSemaphores before it.
# ---------------------------------------------------------------------------
def _split_waits(js):
    for fn in js.get("functions", []):
        for bb in fn.get("blocks", []):
            out = []
            for inst in bb.get("instructions", []):
                si = inst.get("sync_info")
                waits = (si or {}).get("on_wait") or []
                if len(waits) > 1:
                    for k, w in enumerate(waits[:-1]):
                        out.append({
                            "debug": inst.get("debug", 0),
                            "engine": inst["engine"],
                            "ins": [], "outs": [],
                            "name": f"{inst['name']}-wsplit{k}",
                            "opcode": "EventSemaphore",
                            "sync_info": {"on_update": [], "on_wait": [w]},
                        })
                    si["on_wait"] = waits[-1:]
                out.append(inst)
            bb["instructions"] = out
    return js


def _install_patches():
    if getattr(bass_utils.compile_bir_kernel, "_wait_split_patched", False):
        return
    orig = bass_utils.compile_bir_kernel

    def wrapper(bir_json, tmpdir, neff_name="file.neff"):
        js = _split_waits(json.loads(bir_json))
        return orig(json.dumps(js).encode(), tmpdir, neff_name=neff_name)

    wrapper._wait_split_patched = True
    bass_utils.compile_bir_kernel = wrapper
    bass2jax.compile_bir_kernel = wrapper


def _to_f32r(x):
    u = np.ascontiguousarray(x, dtype=np.float32).view(np.uint32).astype(np.uint64)
    lsb = (u >> 12) & 1
    r = (u + 0x7FF + lsb) & np.uint64(0xFFFFF000)
    return r.astype(np.uint32).view(np.float32)


def _n_chunks(vc, nmax=512):
    out, o = [], 0
    while o < vc:
        w = min(nmax, vc - o)
        out.append((o, w))
        o += w
    return out


def _build_kernel(reps=1):
    nc = bass.Bass()
    embT_d = nc.declare_dram_parameter("embT", [128, KE, B], F32, isOutput=False)
    w1T_d = nc.declare_dram_parameter("w1T", [128, KH, 1024], F32, isOutput=False)
    b1c_d = nc.declare_dram_parameter("b1c", [128, KH], F32, isOutput=False)
    w2T_d = nc.declare_dram_parameter("w2T", [128, KE + KH, VC], F32, isOutput=False)
    out_d = nc.declare_dram_parameter("out", [B, VC], F32, isOutput=True)
    MT = CHUNK // 128

    with tile.TileContext(nc) as tc:
        with tc.tile_pool(name="constp", bufs=1) as constp, \
             tc.tile_pool(name="embp", bufs=2) as embp, \
             tc.tile_pool(name="hp", bufs=1) as hp, \
             tc.tile_pool(name="w2p", bufs=2) as w2p, \
             tc.tile_pool(name="outp", bufs=4) as outp, \
             tc.tile_pool(name="psA", bufs=2, space="PSUM") as psA, \
             tc.tile_pool(name="psB", bufs=4, space="PSUM") as psB:

            w1 = constp.tile([128, KH, 1024], F32R)
            nc.sync.dma_start(w1[:], w1T_d[:].bitcast(F32R))
            b1 = constp.tile([128, KH], F32)
            nc.sync.dma_start(b1[:], b1c_d[:])

            for c in range(reps * (B // CHUNK)):
                c = c % (B // CHUNK)
                cb = c * CHUNK
                emb = embp.tile([128, KE, CHUNK], F32R, tag="emb")
                nc.sync.dma_start(emb[:], embT_d[:, :, cb:cb + CHUNK].bitcast(F32R))

                h = hp.tile([128, KH, CHUNK], F32R, tag="h")
                for jt in range(KH):
                    for bh in range(CHUNK // 512):
                        bs = slice(bh * 512, (bh + 1) * 512)
                        ps = psA.tile([128, 512], F32, tag="psA")
                        for k in range(KE):
                            nc.tensor.matmul(
                                ps[:], w1[:, k, jt * 128:(jt + 1) * 128],
                                emb[:, k, bs],
                                start=(k == 0), stop=(k == KE - 1))
                        nc.scalar.activation(
                            h[:, jt, bs], ps[:],
                            mybir.ActivationFunctionType.Tanh,
                            bias=b1[:, jt:jt + 1])

                for (nb, nw) in _n_chunks(VC):
                    w2 = w2p.tile([128, KE + KH, 512], F32R, tag="w2")
                    nc.sync.dma_start(
                        w2[:, :, :nw], w2T_d[:, :, nb:nb + nw].bitcast(F32R))
                    for m in range(MT):
                        ms = slice(m * 128, (m + 1) * 128)
                        ps = psB.tile([128, 512], F32, tag="psB")
                        for k in range(KE + KH):
                            lhsT = emb[:, k, ms] if k < KE else h[:, k - KE, ms]
                            nc.tensor.matmul(
                                ps[:, :nw], lhsT, w2[:, k, :nw],
                                start=(k == 0), stop=(k == KE + KH - 1))
                        o = outp.tile([128, 512], F32, tag="o")
                        nc.vector.tensor_copy(o[:, :nw], ps[:, :nw])
                        nc.sync.dma_start(
                            out_d[cb + m * 128: cb + (m + 1) * 128, nb:nb + nw],
                            o[:, :nw])
    return nc


def host_prep(contexts, W_e, b_e, W_1, b_1, W_2, b_2):
    W_eb = np.asarray(W_e, np.float32) + np.asarray(b_e, np.float32)[:, None]
    emb = W_eb.T[np.asarray(contexts).reshape(-1)]
    embT = _to_f32r(np.ascontiguousarray(
        emb.reshape(B, CTX, EMB).transpose(2, 1, 0)))

    W1p = np.zeros((1024, CTX * EMB), np.float32)
    W1p[:HID] = np.asarray(W_1, np.float32)
    w1T = _to_f32r(np.ascontiguousarray(
        W1p.T.reshape(CTX, EMB, 1024).transpose(1, 0, 2)))
    b1p = np.zeros(1024, np.float32)
    b1p[:HID] = np.asarray(b_1, np.float32)
    b1p[1023] = 20.0
    b1c = np.ascontiguousarray(b1p.reshape(8, 128).T)

    VPAD = VC * N_CORES
    W2p = np.zeros((VPAD, 2048), np.float32)
    W2p[:VOCAB, 0:CTX * EMB] = np.asarray(W_2, np.float32)[:, HID:]
    W2p[:VOCAB, CTX * EMB:CTX * EMB + HID] = np.asarray(W_2, np.float32)[:, :HID]
    W2p[:VOCAB, 2047] = np.asarray(b_2, np.float32)
    W2pT = _to_f32r(np.ascontiguousarray(W2p.T))

    in_maps = []
    for c in range(N_CORES):
        w2cT = np.ascontiguousarray(
            W2pT[:, c * VC:(c + 1) * VC].reshape(16, 128, VC).transpose(1, 0, 2))
        in_maps.append({"embT": embT, "w1T": w1T, "b1c": b1c, "w2T": w2cT})
    return in_maps


_NC_CACHE = {}


def get_nc(reps=1):
    key = ("nc", reps)
    if key not in _NC_CACHE:
        _install_patches()
        _NC_CACHE[key] = _build_kernel(reps)
    return _NC_CACHE[key]


def kernel(contexts, W_e, b_e, W_1, b_1, W_2, b_2):
    nc = get_nc()
    in_maps = host_prep(contexts, W_e, b_e, W_1, b_1, W_2, b_2)
    res = bass_utils.run_bass_kernel_spmd(nc, in_maps, list(range(N_CORES)))
    full = np.concatenate([res.results[c]["out"] for c in range(N_CORES)], axis=1)
    return np.ascontiguousarray(full[:, :VOCAB])
